# revision 1
# baseline (speedup 1.0000x reference)
"""CNN+GRU kernel for Trainium2, 8-core SPMD, data-parallel over batch.

Model (per reference):
  onehot(x) -> Conv1d(V=512,H=512,k=3,pad=1) -> ReLU -> GRU(H=512) -> last
  hidden -> Linear(H,C=20).   x: (B=128, L=1024) int64.

Strategy per core (batch shard of 16):
  Phase A (parallel over L): conv is computed as one-hot matmuls directly in
    transposed layout yT (h on partitions, positions on free dim, l-major
    within the shard so a conv tap shift is a 16-column shift).  Then
    gi = y @ w_ih.T + biases is computed per gate-chunk (gates on psum
    partitions) and streamed to DRAM in scan-friendly layout
    (t, gate_chunk, 128, batch).
  Phase B (sequential scan over 1024 steps): w_hh stationary bf16 matmuls
    (48 per step, gates on partitions, batch=16 on free dim), elementwise
    on DVE/ACT in (128, chunks*16) layout; h kept both f32 (update math)
    and bf16 (next matmul rhs).  gi is prefetched in 32-step blocks via a
    2-stage software pipeline.
  Phase C: classifier matmul in f32r.
"""

import os
from contextlib import ExitStack

import numpy as np
import ml_dtypes

import concourse.bass as bass
import concourse.mybir as mybir
import concourse.tile as tile
from concourse import bacc
from concourse.bass import ds
from concourse.bass_utils import run_bass_kernel_spmd

F32 = mybir.dt.float32
F32R = mybir.dt.float32r
BF16 = mybir.dt.bfloat16

B, L, V, H, C = 128, 1024, 512, 512, 20
NCORES = 8
BS = B // NCORES          # 16 batch rows per core
LBLK = 32                 # timesteps per phase-A chunk (=> 512 positions)
SCAN_BLK = 32             # timesteps per scan pipeline tick

Relu = mybir.ActivationFunctionType.Relu
Identity = mybir.ActivationFunctionType.Identity
Sigmoid = mybir.ActivationFunctionType.Sigmoid
Tanh = mybir.ActivationFunctionType.Tanh
EQ = mybir.AluOpType.is_equal


def build(l_total: int = L):
    nchunk = l_total // LBLK
    nblocks = l_total // SCAN_BLK
    xpad_len = (l_total + 2) * BS

    nc = bacc.Bacc(
        "TRN2", target_bir_lowering=False, debug=False, num_devices=NCORES
    )

    def din(name, shape, dt=F32):
        return nc.dram_tensor(name, list(shape), dt, kind="ExternalInput").ap()

    xpad_d = din("xpad", [xpad_len])                 # l-major, sentinel rows
    wt_d = din("wt", [128, 12, 512], F32R)                 # conv taps (p,[k,vc],h)
    wih_d = din("wih", [128, 4, 3 * H], F32R)              # (p, hc, g)
    whh_d = din("whh", [128, 4, 3 * H], BF16)        # (p, hc, g)
    gib_d = din("gib", [128, 12])                    # b_ih (+b_hh for rz)
    bhn_d = din("bhn", [128, 4, BS])                 # b_hh n-part bcast
    convb_d = din("convb", [128, 4])
    clsw_d = din("clsw", [128, 4, C], BF16)
    clsb_d = din("clsb", [BS, C])
    iota_d = din("iota", [128, 4])
    gi_d = nc.dram_tensor("gi_dram", [l_total, 12, 128, BS], F32).ap()
    out_d = nc.dram_tensor("out", [BS, C], F32, kind="ExternalOutput").ap()

    ET = mybir.EngineType
    hint = (ET.PE, ET.DVE, ET.Activation, ET.SP)

    with tile.TileContext(nc) as tc, ExitStack() as ctx:
        singles = ctx.enter_context(tc.tile_pool(name="singles", bufs=1))

        def load_const(ap_d, name, dt=None):
            t = singles.tile(list(ap_d.shape), dt or ap_d.dtype, tag=name)
            nc.sync.dma_start(t, ap_d)
            return t

        wt_sb = load_const(wt_d, "wt")
        wih_sb = load_const(wih_d, "wih")
        whh_sb = load_const(whh_d, "whh")
        gib_sb = load_const(gib_d, "gib")
        bhn_sb = load_const(bhn_d, "bhn")
        convb_sb = load_const(convb_d, "convb")
        clsw_sb = load_const(clsw_d, "clsw")
        clsb_sb = load_const(clsb_d, "clsb")
        iota_sb = load_const(iota_d, "iota")

        # ---------------- Phase A: conv + gi precompute ----------------
        ctxA = ctx.enter_context(ExitStack())
        ohp = ctxA.enter_context(tc.tile_pool(name="oh", bufs=2))
        ytp = ctxA.enter_context(tc.tile_pool(name="yt", bufs=2))
        gip = ctxA.enter_context(tc.tile_pool(name="gis", bufs=3))
        psA = ctxA.enter_context(tc.tile_pool(name="psA", bufs=4, space="PSUM"))

        w_cols = LBLK * BS + 2 * BS  # 544: 512 positions + halo
        for c in range(nchunk):
            xb = ohp.tile([128, w_cols], F32, tag="xb")
            nc.gpsimd.dma_start(
                xb,
                xpad_d[c * LBLK * BS : c * LBLK * BS + w_cols]
                .partition_broadcast(128),
            )
            ohs = []
            for vc in range(4):
                oh = ohp.tile([128, w_cols], F32R, tag=f"oh{vc}")
                nc.vector.tensor_scalar(
                    oh, xb, iota_sb[:, vc : vc + 1], None, EQ
                )
                ohs.append(oh)
            yts = []
            for m in range(4):
                ps = psA.tile([128, 512], F32, tag="psA")
                n_mm = 0
                for k in range(3):
                    for vc in range(4):
                        nc.tensor.matmul(
                            ps,
                            wt_sb[:, k * 4 + vc, m * 128 : (m + 1) * 128],
                            ohs[vc][:, k * BS : k * BS + 512],
                            start=(n_mm == 0),
                            stop=(n_mm == 11),
                        )
                        n_mm += 1
                yt = ytp.tile([128, 512], F32R, tag=f"yt{m}")
                nc.scalar.activation(yt, ps, Relu, bias=convb_sb[:, m : m + 1])
                yts.append(yt)
            for g in range(12):
                ps = psA.tile([128, 512], F32, tag="psA")
                for hc in range(4):
                    nc.tensor.matmul(
                        ps,
                        wih_sb[:, hc, g * 128 : (g + 1) * 128],
                        yts[hc],
                        start=(hc == 0),
                        stop=(hc == 3),
                    )
                gis = gip.tile([128, 512], F32, tag="gis")
                nc.scalar.activation(gis, ps, Identity, bias=gib_sb[:, g : g + 1])
                nc.sync.dma_start(
                    gi_d[c * LBLK : (c + 1) * LBLK, g].transpose([1, 0, 2]),
                    gis.rearrange("p (l b) -> p l b", b=BS),
                )

        ctxA.close()

        # ---------------- Phase B: GRU scan ----------------
        ctxB = ctx.enter_context(ExitStack())
        scn = ctx.enter_context(tc.tile_pool(name="scn", bufs=3))
        hp = ctx.enter_context(tc.tile_pool(name="hp", bufs=1))
        psRZ = ctxB.enter_context(tc.tile_pool(name="psRZ", bufs=2, space="PSUM"))
        psN = ctxB.enter_context(tc.tile_pool(name="psN", bufs=2, space="PSUM"))

        h32 = hp.tile([128, 4, BS], F32)
        hbf = hp.tile([128, 4, BS], BF16)
        nc.vector.memset(h32, 0.0)
        nc.vector.memset(hbf, 0.0)

        def load_stage(pipe, iv):
            gt = pipe.intermediate_tile([128, SCAN_BLK, 12, BS], F32)
            nc.sync.dma_start(gt, gi_d[ds(iv, SCAN_BLK)].transpose([2, 0, 1, 3]))
            return gt

        def compute_stage(pipe, iv, gt):
            for s in range(SCAN_BLK):
                gin = gt[:, s]  # (128, 12, BS)
                prz = psRZ.tile([128, 8, BS], F32, tag="przt")
                pn = psN.tile([128, 4, BS], F32, tag="pnt")
                for g in range(8):
                    for hc in range(4):
                        nc.tensor.matmul(
                            prz[:, g],
                            whh_sb[:, hc, g * 128 : (g + 1) * 128],
                            hbf[:, hc],
                            start=(hc == 0),
                            stop=(hc == 3),
                        )
                for g in range(8, 12):
                    for hc in range(4):
                        nc.tensor.matmul(
                            pn[:, g - 8],
                            whh_sb[:, hc, g * 128 : (g + 1) * 128],
                            hbf[:, hc],
                            start=(hc == 0),
                            stop=(hc == 3),
                        )
                pre = scn.tile([128, 8, BS], F32, tag="pre")
                nc.vector.tensor_add(pre, gin[:, 0:8], prz)
                rzs = scn.tile([128, 8, BS], F32, tag="rzs")
                nc.scalar.activation(rzs, pre, Sigmoid)
                u = scn.tile([128, 4, BS], F32, tag="u")
                nc.vector.tensor_add(u, pn, bhn_sb)
                v = scn.tile([128, 4, BS], F32, tag="v")
                nc.vector.tensor_mul(v, u, rzs[:, 0:4])
                w = scn.tile([128, 4, BS], F32, tag="w")
                nc.vector.tensor_add(w, v, gin[:, 8:12])
                nt = scn.tile([128, 4, BS], F32, tag="nt")
                nc.scalar.activation(nt, w, Tanh)
                s1 = scn.tile([128, 4, BS], F32, tag="s1")
                nc.vector.tensor_sub(s1, h32, nt)
                s2 = scn.tile([128, 4, BS], F32, tag="s2")
                nc.vector.tensor_mul(s2, rzs[:, 4:8], s1)
                nc.vector.tensor_add(h32, nt, s2)
                nc.vector.tensor_copy(hbf, h32)

        tc.For_i_pipelined(
            [load_stage, compute_stage],
            0,
            l_total,
            SCAN_BLK,
            unroll=2,
            hint_engines=hint,
        )

        ctxB.close()

        # ---------------- Phase C: classifier ----------------
        psC = ctx.enter_context(tc.tile_pool(name="psC", bufs=1, space="PSUM"))
        pc = psC.tile([BS, C], F32)
        for hc in range(4):
            nc.tensor.matmul(
                pc,
                hbf[:, hc],
                clsw_sb[:, hc],
                start=(hc == 0),
                stop=(hc == 3),
            )
        outs = singles.tile([BS, C], F32)
        nc.vector.tensor_add(outs, pc, clsb_sb)
        nc.sync.dma_start(out_d, outs)

    nc.compile()
    return nc


def host_prep(x, conv_w, conv_b, w_ih, w_hh, b_ih, b_hh, cls_w, cls_b,
              l_total: int = L):
    """Build per-core in_maps.  Only cheap O(B*L + V*H) numpy work."""
    x = np.asarray(x)
    conv_w = np.asarray(conv_w, np.float32)
    conv_b = np.asarray(conv_b, np.float32)
    w_ih = np.asarray(w_ih, np.float32)
    w_hh = np.asarray(w_hh, np.float32)
    b_ih = np.asarray(b_ih, np.float32)
    b_hh = np.asarray(b_hh, np.float32)
    cls_w = np.asarray(cls_w, np.float32)
    cls_b = np.asarray(cls_b, np.float32)

    # conv taps: wt[p, k*4+vc, h] = conv_w[h, vc*128+p, k]
    Wv = conv_w.transpose(1, 0, 2)                    # (V, H, 3)
    wt = np.ascontiguousarray(
        Wv.reshape(4, 128, H, 3).transpose(1, 3, 0, 2).reshape(128, 12, H)
    )
    wih = np.ascontiguousarray(
        w_ih.T.reshape(4, 128, 3 * H).transpose(1, 0, 2)
    )
    whh = np.ascontiguousarray(
        w_hh.T.reshape(4, 128, 3 * H).transpose(1, 0, 2)
    ).astype(ml_dtypes.bfloat16)
    bb = b_ih.copy()
    bb[: 2 * H] += b_hh[: 2 * H]
    gib = np.ascontiguousarray(bb.reshape(12, 128).T)
    bhn = np.ascontiguousarray(
        np.repeat(b_hh[2 * H :].reshape(4, 128).T[:, :, None], BS, axis=2)
    )
    convb = np.ascontiguousarray(conv_b.reshape(4, 128).T)
    clsw = np.ascontiguousarray(cls_w.T.reshape(4, 128, C).transpose(1, 0, 2)).astype(ml_dtypes.bfloat16)
    clsb = np.tile(cls_b[None, :], (BS, 1)).astype(np.float32)
    iota = np.ascontiguousarray(
        np.arange(V, dtype=np.float32).reshape(4, 128).T
    )

    shared = {
        "wt": wt, "wih": wih, "whh": whh, "gib": gib, "bhn": bhn,
        "convb": convb, "clsw": clsw, "clsb": clsb, "iota": iota,
    }
    in_maps = []
    for c in range(NCORES):
        xc = x[c * BS : (c + 1) * BS, :l_total].astype(np.float32).T  # (l, BS)
        xpad = np.full((l_total + 2, BS), float(V), np.float32)
        xpad[1:-1] = xc
        in_maps.append({**shared, "xpad": np.ascontiguousarray(xpad.ravel())})
    return in_maps


_BUILT = {}


def _get_nc(l_total: int = L):
    if l_total not in _BUILT:
        _BUILT[l_total] = build(l_total)
    return _BUILT[l_total]


def kernel(x, conv_w, conv_b, w_ih, w_hh, b_ih, b_hh, cls_w, cls_b):
    nc = _get_nc(L)
    in_maps = host_prep(
        x, conv_w, conv_b, w_ih, w_hh, b_ih, b_hh, cls_w, cls_b
    )
    res = run_bass_kernel_spmd(nc, in_maps, core_ids=list(range(NCORES)))
    out = np.concatenate([res.results[c]["out"] for c in range(NCORES)], axis=0)
    return out.astype(np.float32)



# revision 8
# speedup vs baseline: 1.2118x; 1.2118x over previous
"""CNN+GRU kernel for Trainium2, 8-core SPMD, data-parallel over batch.

Model (per reference):
  onehot(x) -> Conv1d(V=512,H=512,k=3,pad=1) -> ReLU -> GRU(H=512) -> last
  hidden -> Linear(H,C=20).   x: (B=128, L=1024) int64.

Fused single-pass design (per core, batch shard of 16):
  The sequence is processed in 32 chunks of 32 timesteps.  For chunk c the
  GRU scan (serial, latency-bound) runs while the conv+gi precompute
  ("phase A") for chunk c+1 is instruction-interleaved into the same
  engines' idle slots; gi never leaves SBUF (double-buffered per parity).
  This removes the 200MB gi DRAM roundtrip of the two-pass version and
  keeps the PE array busy enough to hold its HAM clock at 2.4 GHz.

  Scan step (all gates bf16 matmuls, h kept in bf16 only):
    - one identity matmul preloads all 8 r/z gi slices into PSUM (start)
    - one K=4 selector matmul preloads b_hh[n] into the n-gate PSUM
    - 48 whh matmul pairs accumulate W_hh @ h (r chunks, n chunks, z chunks)
    - ACT: sigmoid(r) [psum], sigmoid(z) [psum], tanh(w)
    - DVE: v=pn*r, w=v+gi_n, omz=1-z, zh=z*h, m=nt*omz, h'=m+zh (bf16 out)
  Phase A per chunk: one-hot on GPSIMD, 48 conv + 48 gi matmuls on PE
  (3-4 per scan step), ReLU on ACT, psum->SBUF gi copies (+bias) on GPSIMD.
"""

import numpy as np
import ml_dtypes
from contextlib import ExitStack

import concourse.bass as bass
import concourse.mybir as mybir
import concourse.tile as tile
from concourse import bacc
from concourse.bass import ds
from concourse.bass_utils import run_bass_kernel_spmd

F32 = mybir.dt.float32
BF16 = mybir.dt.bfloat16

B, L, V, H, C = 128, 1024, 512, 512, 20
NCORES = 8
BS = B // NCORES          # 16 batch rows per core
LBLK = 32                 # timesteps per chunk
NCHUNK = L // LBLK        # 32 chunks
W_COLS = LBLK * BS + 2 * BS   # 544: 512 positions + conv halo

Relu = mybir.ActivationFunctionType.Relu
Identity = mybir.ActivationFunctionType.Identity
Sigmoid = mybir.ActivationFunctionType.Sigmoid
Tanh = mybir.ActivationFunctionType.Tanh
EQ = mybir.AluOpType.is_equal
ADD = mybir.AluOpType.add
MULT = mybir.AluOpType.mult


def build():
    nc = bacc.Bacc(
        "TRN2", target_bir_lowering=False, debug=False, num_devices=NCORES
    )

    def din(name, shape, dt=F32):
        return nc.dram_tensor(name, list(shape), dt, kind="ExternalInput").ap()

    xpad_d = din("xpad", [(L + 2) * BS])            # l-major, sentinel rows
    wt_d = din("wt", [128, 12, 512], BF16)          # conv taps (p,[k,vc],h)
    wih_d = din("wih", [128, 4, 3 * H], BF16)       # (p, hc, g)
    whh_d = din("whh", [128, 4, 3 * H], BF16)       # (p, hc, g)
    gib_d = din("gib", [128, 12])                   # b_ih (+b_hh for r,z)
    bhnT_d = din("bhnT", [4, 128], BF16)            # b_hh n-part, [hc, p]
    sel_d = din("sel", [4, 64], BF16)               # selector for bhn bcast
    ident_d = din("ident", [128, 128], BF16)        # identity for gi preload
    convb_d = din("convb", [128, 4])
    clsw_d = din("clsw", [128, 4, C], BF16)
    clsb_d = din("clsb", [BS, C])
    iota_d = din("iota", [128, 4])
    out_d = nc.dram_tensor("out", [BS, C], F32, kind="ExternalOutput").ap()

    ET = mybir.EngineType
    hint = (ET.PE, ET.DVE, ET.Activation, ET.SP, ET.Pool)

    with tile.TileContext(nc) as tc, ExitStack() as ctx:
        singles = ctx.enter_context(tc.tile_pool(name="singles", bufs=1))

        def load_const(ap_d, name):
            t = singles.tile(list(ap_d.shape), ap_d.dtype, tag=name)
            nc.sync.dma_start(t, ap_d)
            return t

        wt_sb = load_const(wt_d, "wt")
        wih_sb = load_const(wih_d, "wih")
        whh_sb = load_const(whh_d, "whh")
        gib_sb = load_const(gib_d, "gib")
        bhnT_sb = load_const(bhnT_d, "bhnT")
        sel_sb = load_const(sel_d, "sel")
        ident_sb = load_const(ident_d, "ident")
        convb_sb = load_const(convb_d, "convb")
        clsw_sb = load_const(clsw_d, "clsw")
        clsb_sb = load_const(clsb_d, "clsb")
        iota_sb = load_const(iota_d, "iota")

        # ---- persistent state / double-buffered tiles (parity-indexed) ----
        state = ctx.enter_context(tc.tile_pool(name="state", bufs=1))
        # h in bf16 only, double-buffered per step parity
        hbf = [state.tile([128, 4, BS], BF16, name=f"hbf{i}") for i in range(2)]
        # gi buffers per chunk parity: r/z part bf16 (matmul rhs), n part f32
        girz = [state.tile([128, 8, 512], BF16, name=f"girz{i}") for i in range(2)]
        gin = [state.tile([128, 4, 512], F32, name=f"gin{i}") for i in range(2)]
        # phase-A transients, per chunk parity
        xb = [state.tile([128, W_COLS], F32, name=f"xb{i}") for i in range(2)]
        oh = [[state.tile([128, W_COLS], BF16, name=f"oh{i}_{vc}") for vc in range(4)]
              for i in range(2)]
        yt = [[state.tile([128, 512], BF16, name=f"yt{i}_{m}") for m in range(4)]
              for i in range(2)]
        # scan elementwise scratch
        rs = state.tile([128, 4, BS], F32)
        zs = state.tile([128, 4, BS], F32)
        omz = state.tile([128, 4, BS], F32)
        zh = state.tile([128, 4, BS], F32)
        vv = state.tile([128, 4, BS], F32)
        ww = state.tile([128, 4, BS], F32)
        nt = state.tile([128, 4, BS], F32)

        # PSUM pools
        psScan = ctx.enter_context(tc.tile_pool(name="psScan", bufs=2, space="PSUM"))
        psConv = ctx.enter_context(tc.tile_pool(name="psConv", bufs=2, space="PSUM"))
        psGi = ctx.enter_context(tc.tile_pool(name="psGi", bufs=2, space="PSUM"))

        nc.vector.memset(hbf[0], 0.0)

        # ---------------- phase A op-closure generator ----------------
        # Yields ("pe"|"other", closure).  "pe" closures emit tensor-engine
        # matmuls and are placed at the BOTTOM of a scan step (after the
        # step's scan matmuls, filling the PE-idle tail); "other" closures
        # emit ACT/GPSIMD/DMA ops and go at the TOP of a step so they never
        # sit in front of the step's critical ACT work in queue order.
        # FIFO order is preserved so producers always precede consumers.
        def phase_a_ops(p, xoff):
            def dma_x():
                nc.sync.dma_start(
                    xb[p], xpad_d[ds(xoff, W_COLS)].partition_broadcast(128)
                )
            yield "other", dma_x
            for vc in range(4):
                def onehot(vc=vc):
                    nc.gpsimd.tensor_scalar(
                        oh[p][vc], xb[p], iota_sb[:, vc : vc + 1], None, EQ
                    )
                yield "other", onehot
            # conv: 4 output chunks x 12 accum matmuls, 3 per closure
            conv_ps = [None] * 4
            for m in range(4):
                for part in range(4):
                    def conv_mm(m=m, part=part):
                        if part == 0:
                            conv_ps[m] = psConv.tile([128, 512], F32, name="cps", tag="cps")
                        for i in range(3):
                            j = part * 3 + i
                            k, vc = j // 4, j % 4
                            nc.tensor.matmul(
                                conv_ps[m],
                                wt_sb[:, k * 4 + vc, m * 128 : (m + 1) * 128],
                                oh[p][vc][:, k * BS : k * BS + 512],
                                start=(j == 0),
                                stop=(j == 11),
                            )
                    yield "pe", conv_mm
                def relu(m=m):
                    nc.scalar.activation(
                        yt[p][m], conv_ps[m], Relu, bias=convb_sb[:, m : m + 1]
                    )
                yield "other", relu
            # gi: 12 gate chunks x 4 accum matmuls + gpsimd copy w/ bias
            gi_ps = [None] * 12
            for g in range(12):
                def gi_mm(g=g):
                    gi_ps[g] = psGi.tile([128, 512], F32, name="gps", tag="gps")
                    for hc in range(4):
                        nc.tensor.matmul(
                            gi_ps[g],
                            wih_sb[:, hc, g * 128 : (g + 1) * 128],
                            yt[p][hc],
                            start=(hc == 0),
                            stop=(hc == 3),
                        )
                yield "pe", gi_mm
                def gi_copy(g=g):
                    if g < 8:
                        dst = girz[p][:, g]
                    else:
                        dst = gin[p][:, g - 8]
                    nc.scalar.activation(
                        dst, gi_ps[g], Identity, bias=gib_sb[:, g : g + 1]
                    )
                yield "other", gi_copy

        class PADispatch:
            """Strict-FIFO dispatcher: per scan step, emit leading 'other'
            closures at the step top (max 2) and then, after the scan
            matmuls, leading 'pe' closures (max 1)."""
            def __init__(self, gen):
                self.pend = list(gen) if gen is not None else []
                self.i = 0

            def top(self):
                n = 0
                while self.i < len(self.pend) and n < 2 \
                        and self.pend[self.i][0] == "other":
                    self.pend[self.i][1]()
                    self.i += 1
                    n += 1

            def bottom(self):
                if self.i < len(self.pend) and self.pend[self.i][0] == "pe":
                    self.pend[self.i][1]()
                    self.i += 1

            def drain(self):
                while self.i < len(self.pend):
                    self.pend[self.i][1]()
                    self.i += 1

        # ---------------- scan step ----------------
        def scan_step(p_gi, s, hb_in, hb_out, pa):
            pa.top()
            pst = psScan.tile([128, 12, BS], F32, name="pst", tag="pst")
            # gi preload: one matmul copies all 8 r/z gi slices into psum
            nc.tensor.matmul(
                pst[:, 0:8], ident_sb, girz[p_gi][:, :, s * BS : (s + 1) * BS],
                start=True, stop=False,
            )
            # bhn preload via K=4 selector
            nc.tensor.matmul(pst[:, 8:12], bhnT_sb, sel_sb, start=True, stop=False)
            # whh accumulation: r chunks, n chunks, z chunks
            for g in (0, 1, 2, 3, 8, 9, 10, 11, 4, 5, 6, 7):
                for hc in range(4):
                    nc.tensor.matmul(
                        pst[:, g],
                        whh_sb[:, hc, g * 128 : (g + 1) * 128],
                        hb_in[:, hc],
                        start=False,
                        stop=(hc == 3),
                    )
            # interleave phase-A PE work for the next chunk into this step
            pa.bottom()
            # elementwise chain
            nc.scalar.activation(rs, pst[:, 0:4], Sigmoid)
            nc.vector.tensor_mul(vv, pst[:, 8:12], rs)
            nc.vector.tensor_add(ww, vv, gin[p_gi][:, :, s * BS : (s + 1) * BS])
            nc.scalar.activation(zs, pst[:, 4:8], Sigmoid)
            nc.scalar.activation(nt, ww, Tanh)
            nc.vector.tensor_scalar(omz, zs, -1.0, 1.0, MULT, ADD)  # 1-z
            nc.vector.tensor_mul(zh, zs, hb_in)
            nc.vector.tensor_mul(vv, nt, omz)          # reuse vv as m
            nc.vector.tensor_add(hb_out, vv, zh)       # bf16 h'

        def scan_chunk(c_par, pa):
            for s in range(LBLK):
                scan_step(c_par, s, hbf[s % 2], hbf[1 - s % 2], pa)

        # ---------------- prologue: phase A for chunk 0 ----------------
        PADispatch(phase_a_ops(0, 0)).drain()

        # ---------------- main loop ----------------
        # chunk pair (2t, 2t+1); phase A for chunks 2t+1, 2t+2.
        # trips cover scan chunks 0..29, phase A 1..30;
        # tail: scan 30 + phase A 31, then scan 31.
        with tc.For_i(0, 30, 2, hint_engines=hint) as t2:
            paA = PADispatch(phase_a_ops(1, (t2 + 1) * (LBLK * BS)))
            scan_chunk(0, paA)
            paA.drain()
            paB = PADispatch(phase_a_ops(0, (t2 + 2) * (LBLK * BS)))
            scan_chunk(1, paB)
            paB.drain()

        # tail: scan chunk 30 (parity 0), phase A chunk 31 -> buf 1
        paT = PADispatch(phase_a_ops(1, 31 * (LBLK * BS)))
        scan_chunk(0, paT)
        paT.drain()
        # scan chunk 31 (parity 1), no phase A
        scan_chunk(1, PADispatch(None))

        # ---------------- classifier ----------------
        psC = ctx.enter_context(tc.tile_pool(name="psC", bufs=1, space="PSUM"))
        pc = psC.tile([BS, C], F32)
        for hc in range(4):
            nc.tensor.matmul(
                pc, hbf[0][:, hc], clsw_sb[:, hc],
                start=(hc == 0), stop=(hc == 3),
            )
        outs = singles.tile([BS, C], F32)
        nc.vector.tensor_add(outs, pc, clsb_sb)
        nc.sync.dma_start(out_d, outs)

    nc.compile()
    return nc


def host_prep(x, conv_w, conv_b, w_ih, w_hh, b_ih, b_hh, cls_w, cls_b):
    x = np.asarray(x)
    conv_w = np.asarray(conv_w, np.float32)
    conv_b = np.asarray(conv_b, np.float32)
    w_ih = np.asarray(w_ih, np.float32)
    w_hh = np.asarray(w_hh, np.float32)
    b_ih = np.asarray(b_ih, np.float32)
    b_hh = np.asarray(b_hh, np.float32)
    cls_w = np.asarray(cls_w, np.float32)
    cls_b = np.asarray(cls_b, np.float32)
    bf16 = ml_dtypes.bfloat16

    # conv taps: wt[p, k*4+vc, h] = conv_w[h, vc*128+p, k]
    Wv = conv_w.transpose(1, 0, 2)                    # (V, H, 3)
    wt = np.ascontiguousarray(
        Wv.reshape(4, 128, H, 3).transpose(1, 3, 0, 2).reshape(128, 12, H)
    ).astype(bf16)
    wih = np.ascontiguousarray(
        w_ih.T.reshape(4, 128, 3 * H).transpose(1, 0, 2)
    ).astype(bf16)
    whh = np.ascontiguousarray(
        w_hh.T.reshape(4, 128, 3 * H).transpose(1, 0, 2)
    ).astype(bf16)
    bb = b_ih.copy()
    bb[: 2 * H] += b_hh[: 2 * H]
    gib = np.ascontiguousarray(bb.reshape(12, 128).T)
    bhnT = np.ascontiguousarray(b_hh[2 * H :].reshape(4, 128)).astype(bf16)
    sel = np.zeros((4, 4, BS), np.float32)
    for k in range(4):
        sel[k, k, :] = 1.0
    sel = sel.reshape(4, 64).astype(bf16)
    ident = np.eye(128, dtype=np.float32).astype(bf16)
    convb = np.ascontiguousarray(conv_b.reshape(4, 128).T)
    clsw = np.ascontiguousarray(
        cls_w.T.reshape(4, 128, C).transpose(1, 0, 2)
    ).astype(bf16)
    clsb = np.tile(cls_b[None, :], (BS, 1)).astype(np.float32)
    iota = np.ascontiguousarray(np.arange(V, dtype=np.float32).reshape(4, 128).T)

    shared = {
        "wt": wt, "wih": wih, "whh": whh, "gib": gib, "bhnT": bhnT,
        "sel": sel, "ident": ident, "convb": convb, "clsw": clsw,
        "clsb": clsb, "iota": iota,
    }
    in_maps = []
    for c in range(NCORES):
        xc = x[c * BS : (c + 1) * BS, :].astype(np.float32).T  # (L, BS)
        xpad = np.full((L + 2, BS), float(V), np.float32)
        xpad[1:-1] = xc
        in_maps.append({**shared, "xpad": np.ascontiguousarray(xpad.ravel())})
    return in_maps


_BUILT = {}


def _get_nc():
    if "nc" not in _BUILT:
        _BUILT["nc"] = build()
    return _BUILT["nc"]


def kernel(x, conv_w, conv_b, w_ih, w_hh, b_ih, b_hh, cls_w, cls_b):
    nc = _get_nc()
    in_maps = host_prep(
        x, conv_w, conv_b, w_ih, w_hh, b_ih, b_hh, cls_w, cls_b
    )
    res = run_bass_kernel_spmd(nc, in_maps, core_ids=list(range(NCORES)))
    out = np.concatenate([res.results[c]["out"] for c in range(NCORES)], axis=0)
    return out.astype(np.float32)


# revision 11
# speedup vs baseline: 1.4667x; 1.2104x over previous
"""CNN+GRU kernel for Trainium2, 8-core SPMD, data-parallel over batch.

Model (per reference):
  onehot(x) -> Conv1d(V=512,H=512,k=3,pad=1) -> ReLU -> GRU(H=512) -> last
  hidden -> Linear(H,C=20).   x: (B=128, L=1024) int64.

Fused single-pass design (per core, batch shard of 16):
  The sequence is processed in 32 chunks of 32 timesteps.  For chunk c the
  GRU scan (serial, latency-bound) runs while the conv+gi precompute
  ("phase A") for chunk c+1 is instruction-interleaved into the same
  engines' idle slots; gi never leaves SBUF (double-buffered per parity).
  This removes the 200MB gi DRAM roundtrip of the two-pass version and
  keeps the PE array busy enough to hold its HAM clock at 2.4 GHz.

  Scan step (all gates bf16 matmuls, h kept in bf16 only):
    - one identity matmul preloads all 8 r/z gi slices into PSUM (start)
    - one K=4 selector matmul preloads b_hh[n] into the n-gate PSUM
    - 48 whh matmul pairs accumulate W_hh @ h (r chunks, n chunks, z chunks)
    - ACT: sigmoid(r) [psum], sigmoid(z) [psum], tanh(w)
    - DVE: v=pn*r, w=v+gi_n, omz=1-z, zh=z*h, m=nt*omz, h'=m+zh (bf16 out)
  Phase A per chunk: one-hot on GPSIMD, 48 conv + 48 gi matmuls on PE
  (3-4 per scan step), ReLU on ACT, psum->SBUF gi copies (+bias) on GPSIMD.
"""

import numpy as np
import ml_dtypes
from contextlib import ExitStack

import concourse.bass as bass
import concourse.mybir as mybir
import concourse.tile as tile
from concourse import bacc
from concourse.bass import ds
from concourse.bass_utils import run_bass_kernel_spmd

F32 = mybir.dt.float32
BF16 = mybir.dt.bfloat16

B, L, V, H, C = 128, 1024, 512, 512, 20
NCORES = 8
BS = B // NCORES          # 16 batch rows per core
LBLK = 32                 # timesteps per chunk
NCHUNK = L // LBLK        # 32 chunks
W_COLS = LBLK * BS + 2 * BS   # 544: 512 positions + conv halo

Relu = mybir.ActivationFunctionType.Relu
Identity = mybir.ActivationFunctionType.Identity
Sigmoid = mybir.ActivationFunctionType.Sigmoid
Tanh = mybir.ActivationFunctionType.Tanh
EQ = mybir.AluOpType.is_equal
ADD = mybir.AluOpType.add
MULT = mybir.AluOpType.mult


def build():
    nc = bacc.Bacc(
        "TRN2", target_bir_lowering=False, debug=False, num_devices=NCORES
    )

    def din(name, shape, dt=F32):
        return nc.dram_tensor(name, list(shape), dt, kind="ExternalInput").ap()

    xpad_d = din("xpad", [(L + 2) * BS])            # l-major, sentinel rows
    wt_d = din("wt", [128, 12, 512], BF16)          # conv taps (p,[k,vc],h)
    wih_d = din("wih", [128, 4, 3 * H], BF16)       # (p, hc, g)
    whh_d = din("whh", [128, 4, 3 * H], BF16)       # (p, hc, g)
    gib_d = din("gib", [128, 12])                   # b_ih (+b_hh for r,z)
    bhnT_d = din("bhnT", [4, 128], BF16)            # b_hh n-part, [hc, p]
    sel_d = din("sel", [4, 64], BF16)               # selector for bhn bcast
    ident_d = din("ident", [128, 128], BF16)        # identity for gi preload
    convb_d = din("convb", [128, 4])
    clsw_d = din("clsw", [128, 4, C], BF16)
    clsb_d = din("clsb", [BS, C])
    iota_d = din("iota", [128, 4])
    out_d = nc.dram_tensor("out", [BS, C], F32, kind="ExternalOutput").ap()

    ET = mybir.EngineType
    hint = (ET.PE, ET.DVE, ET.Activation, ET.SP, ET.Pool)

    with tile.TileContext(nc) as tc, ExitStack() as ctx:
        singles = ctx.enter_context(tc.tile_pool(name="singles", bufs=1))

        def load_const(ap_d, name):
            t = singles.tile(list(ap_d.shape), ap_d.dtype, tag=name)
            nc.sync.dma_start(t, ap_d)
            return t

        wt_sb = load_const(wt_d, "wt")
        wih_sb = load_const(wih_d, "wih")
        whh_sb = load_const(whh_d, "whh")
        gib_sb = load_const(gib_d, "gib")
        bhnT_sb = load_const(bhnT_d, "bhnT")
        sel_sb = load_const(sel_d, "sel")
        ident_sb = load_const(ident_d, "ident")
        convb_sb = load_const(convb_d, "convb")
        clsw_sb = load_const(clsw_d, "clsw")
        clsb_sb = load_const(clsb_d, "clsb")
        iota_sb = load_const(iota_d, "iota")

        # ---- persistent state / double-buffered tiles (parity-indexed) ----
        state = ctx.enter_context(tc.tile_pool(name="state", bufs=1))
        # h in bf16 only, double-buffered per step parity
        hbf = [state.tile([128, 4, BS], BF16, name=f"hbf{i}") for i in range(2)]
        # gi buffers per chunk parity: r/z part bf16 (matmul rhs), n part f32
        girz = [state.tile([128, 8, 512], BF16, name=f"girz{i}") for i in range(2)]
        gin = [state.tile([128, 4, 512], F32, name=f"gin{i}") for i in range(2)]
        # phase-A transients, per chunk parity
        xb = [state.tile([128, W_COLS], F32, name=f"xb{i}") for i in range(2)]
        oh = [[state.tile([128, W_COLS], BF16, name=f"oh{i}_{vc}") for vc in range(4)]
              for i in range(2)]
        yt = [[state.tile([128, 512], BF16, name=f"yt{i}_{m}") for m in range(4)]
              for i in range(2)]
        # scan elementwise scratch
        rs = state.tile([128, 4, BS], F32)
        zs = state.tile([128, 4, BS], F32)
        omz = state.tile([128, 4, BS], F32)
        zh = state.tile([128, 4, BS], F32)
        vv = state.tile([128, 4, BS], F32)
        ww = state.tile([128, 4, BS], F32)
        nt = state.tile([128, 4, BS], F32)

        # PSUM pools
        psScan = ctx.enter_context(tc.tile_pool(name="psScan", bufs=2, space="PSUM"))
        psConv = ctx.enter_context(tc.tile_pool(name="psConv", bufs=1, space="PSUM"))
        psGi = ctx.enter_context(tc.tile_pool(name="psGi", bufs=1, space="PSUM"))

        nc.vector.memset(hbf[0], 0.0)

        # ---------------- phase A op-closure generator ----------------
        # Yields ("pe"|"other", closure).  "pe" closures emit tensor-engine
        # matmuls and are placed at the BOTTOM of a scan step (after the
        # step's scan matmuls, filling the PE-idle tail); "other" closures
        # emit ACT/GPSIMD/DMA ops and go at the TOP of a step so they never
        # sit in front of the step's critical ACT work in queue order.
        # FIFO order is preserved so producers always precede consumers.
        def phase_a_ops(p, xoff):
            def dma_x():
                nc.sync.dma_start(
                    xb[p], xpad_d[ds(xoff, W_COLS)].partition_broadcast(128)
                )
            yield "other", dma_x
            for vc in range(4):
                def onehot(vc=vc):
                    nc.vector.tensor_scalar(
                        oh[p][vc], xb[p], iota_sb[:, vc : vc + 1], None, EQ
                    )
                yield "other", onehot
            # conv: 4 output chunks x 12 accum matmuls, 3 per closure
            conv_ps = [None] * 4
            for m in range(4):
                for part in range(4):
                    def conv_mm(m=m, part=part):
                        if part == 0:
                            conv_ps[m] = psConv.tile([128, 512], F32, name="cps", tag="cps")
                        for i in range(3):
                            j = part * 3 + i
                            k, vc = j // 4, j % 4
                            nc.tensor.matmul(
                                conv_ps[m],
                                wt_sb[:, k * 4 + vc, m * 128 : (m + 1) * 128],
                                oh[p][vc][:, k * BS : k * BS + 512],
                                start=(j == 0),
                                stop=(j == 11),
                            )
                    yield "pe", conv_mm
                def relu(m=m):
                    nc.scalar.activation(
                        yt[p][m], conv_ps[m], Relu, bias=convb_sb[:, m : m + 1]
                    )
                yield "other", relu
            # gi: 12 gate chunks x 4 accum matmuls + gpsimd copy w/ bias
            gi_ps = [None] * 12
            for g in range(12):
                def gi_mm(g=g):
                    gi_ps[g] = psGi.tile([128, 512], F32, name="gps", tag="gps")
                    for hc in range(4):
                        nc.tensor.matmul(
                            gi_ps[g],
                            wih_sb[:, hc, g * 128 : (g + 1) * 128],
                            yt[p][hc],
                            start=(hc == 0),
                            stop=(hc == 3),
                        )
                yield "pe", gi_mm
                def gi_copy(g=g):
                    if g < 8:
                        dst = girz[p][:, g]
                    else:
                        dst = gin[p][:, g - 8]
                    nc.vector.tensor_scalar(
                        dst, gi_ps[g], gib_sb[:, g : g + 1], None, ADD
                    )
                yield "other", gi_copy

        class PADispatch:
            """Strict-FIFO dispatcher: per scan step, emit leading 'other'
            closures at the step top (max 2) and then, after the scan
            matmuls, leading 'pe' closures (max 1)."""
            def __init__(self, gen):
                self.pend = list(gen) if gen is not None else []
                self.i = 0

            def top(self):
                n = 0
                while self.i < len(self.pend) and n < 2 \
                        and self.pend[self.i][0] == "other":
                    self.pend[self.i][1]()
                    self.i += 1
                    n += 1

            def bottom(self):
                if self.i < len(self.pend) and self.pend[self.i][0] == "pe":
                    self.pend[self.i][1]()
                    self.i += 1

            def drain(self):
                while self.i < len(self.pend):
                    self.pend[self.i][1]()
                    self.i += 1

        # ---------------- scan step ----------------
        def scan_step(p_gi, s, hb_in, hb_out, pa):
            pa.top()
            pr = psScan.tile([128, 4, BS], F32, name="pr", tag="pr")
            pz = psScan.tile([128, 4, BS], F32, name="pz", tag="pz")
            pn = psScan.tile([128, 4, BS], F32, name="pn", tag="pn")
            # gi preloads (no h dependency: run during previous step's tail)
            nc.tensor.matmul(
                pr, ident_sb, girz[p_gi][:, 0:4, s * BS : (s + 1) * BS],
                start=True, stop=False,
            )
            nc.tensor.matmul(
                pz, ident_sb, girz[p_gi][:, 4:8, s * BS : (s + 1) * BS],
                start=True, stop=False,
            )
            # bhn preload via K=4 selector
            nc.tensor.matmul(pn, bhnT_sb, sel_sb, start=True, stop=False)
            # whh accumulation: r chunks, n chunks, z chunks
            for g in (0, 1, 2, 3, 8, 9, 10, 11, 4, 5, 6, 7):
                dst = pr[:, g] if g < 4 else (pz[:, g - 4] if g < 8 else pn[:, g - 8])
                for hc in range(4):
                    nc.tensor.matmul(
                        dst,
                        whh_sb[:, hc, g * 128 : (g + 1) * 128],
                        hb_in[:, hc],
                        start=False,
                        stop=(hc == 3),
                    )
            # interleave phase-A PE work for the next chunk into this step
            pa.bottom()
            # elementwise chain
            nc.scalar.activation(rs, pr, Sigmoid)
            nc.vector.tensor_mul(vv, pn, rs)
            nc.vector.tensor_add(ww, vv, gin[p_gi][:, :, s * BS : (s + 1) * BS])
            nc.scalar.activation(zs, pz, Sigmoid)
            nc.scalar.activation(nt, ww, Tanh)
            nc.vector.tensor_scalar(omz, zs, -1.0, 1.0, MULT, ADD)  # 1-z
            nc.vector.tensor_mul(zh, zs, hb_in)
            nc.vector.tensor_mul(vv, nt, omz)          # reuse vv as m
            nc.vector.tensor_add(hb_out, vv, zh)       # bf16 h'

        def scan_chunk(c_par, pa):
            for s in range(LBLK):
                scan_step(c_par, s, hbf[s % 2], hbf[1 - s % 2], pa)

        # ---------------- prologue: phase A for chunk 0 ----------------
        PADispatch(phase_a_ops(0, 0)).drain()

        # ---------------- main loop ----------------
        # chunk pair (2t, 2t+1); phase A for chunks 2t+1, 2t+2.
        # trips cover scan chunks 0..29, phase A 1..30;
        # tail: scan 30 + phase A 31, then scan 31.
        with tc.For_i(0, 30, 2, hint_engines=hint) as t2:
            paA = PADispatch(phase_a_ops(1, (t2 + 1) * (LBLK * BS)))
            scan_chunk(0, paA)
            paA.drain()
            paB = PADispatch(phase_a_ops(0, (t2 + 2) * (LBLK * BS)))
            scan_chunk(1, paB)
            paB.drain()

        # tail: scan chunk 30 (parity 0), phase A chunk 31 -> buf 1
        paT = PADispatch(phase_a_ops(1, 31 * (LBLK * BS)))
        scan_chunk(0, paT)
        paT.drain()
        # scan chunk 31 (parity 1), no phase A
        scan_chunk(1, PADispatch(None))

        # ---------------- classifier ----------------
        pc = psGi.tile([BS, C], F32, name="pc", tag="gps",
                       padded_shape=[BS, 512])
        for hc in range(4):
            nc.tensor.matmul(
                pc, hbf[0][:, hc], clsw_sb[:, hc],
                start=(hc == 0), stop=(hc == 3),
            )
        outs = singles.tile([BS, C], F32)
        nc.vector.tensor_add(outs, pc, clsb_sb)
        nc.sync.dma_start(out_d, outs)

    nc.compile()
    return nc


def host_prep(x, conv_w, conv_b, w_ih, w_hh, b_ih, b_hh, cls_w, cls_b):
    x = np.asarray(x)
    conv_w = np.asarray(conv_w, np.float32)
    conv_b = np.asarray(conv_b, np.float32)
    w_ih = np.asarray(w_ih, np.float32)
    w_hh = np.asarray(w_hh, np.float32)
    b_ih = np.asarray(b_ih, np.float32)
    b_hh = np.asarray(b_hh, np.float32)
    cls_w = np.asarray(cls_w, np.float32)
    cls_b = np.asarray(cls_b, np.float32)
    bf16 = ml_dtypes.bfloat16

    # conv taps: wt[p, k*4+vc, h] = conv_w[h, vc*128+p, k]
    Wv = conv_w.transpose(1, 0, 2)                    # (V, H, 3)
    wt = np.ascontiguousarray(
        Wv.reshape(4, 128, H, 3).transpose(1, 3, 0, 2).reshape(128, 12, H)
    ).astype(bf16)
    wih = np.ascontiguousarray(
        w_ih.T.reshape(4, 128, 3 * H).transpose(1, 0, 2)
    ).astype(bf16)
    whh = np.ascontiguousarray(
        w_hh.T.reshape(4, 128, 3 * H).transpose(1, 0, 2)
    ).astype(bf16)
    bb = b_ih.copy()
    bb[: 2 * H] += b_hh[: 2 * H]
    gib = np.ascontiguousarray(bb.reshape(12, 128).T)
    bhnT = np.ascontiguousarray(b_hh[2 * H :].reshape(4, 128)).astype(bf16)
    sel = np.zeros((4, 4, BS), np.float32)
    for k in range(4):
        sel[k, k, :] = 1.0
    sel = sel.reshape(4, 64).astype(bf16)
    ident = np.eye(128, dtype=np.float32).astype(bf16)
    convb = np.ascontiguousarray(conv_b.reshape(4, 128).T)
    clsw = np.ascontiguousarray(
        cls_w.T.reshape(4, 128, C).transpose(1, 0, 2)
    ).astype(bf16)
    clsb = np.tile(cls_b[None, :], (BS, 1)).astype(np.float32)
    iota = np.ascontiguousarray(np.arange(V, dtype=np.float32).reshape(4, 128).T)

    shared = {
        "wt": wt, "wih": wih, "whh": whh, "gib": gib, "bhnT": bhnT,
        "sel": sel, "ident": ident, "convb": convb, "clsw": clsw,
        "clsb": clsb, "iota": iota,
    }
    in_maps = []
    for c in range(NCORES):
        xc = x[c * BS : (c + 1) * BS, :].astype(np.float32).T  # (L, BS)
        xpad = np.full((L + 2, BS), float(V), np.float32)
        xpad[1:-1] = xc
        in_maps.append({**shared, "xpad": np.ascontiguousarray(xpad.ravel())})
    return in_maps


_BUILT = {}


def _get_nc():
    if "nc" not in _BUILT:
        _BUILT["nc"] = build()
    return _BUILT["nc"]


def kernel(x, conv_w, conv_b, w_ih, w_hh, b_ih, b_hh, cls_w, cls_b):
    nc = _get_nc()
    in_maps = host_prep(
        x, conv_w, conv_b, w_ih, w_hh, b_ih, b_hh, cls_w, cls_b
    )
    res = run_bass_kernel_spmd(nc, in_maps, core_ids=list(range(NCORES)))
    out = np.concatenate([res.results[c]["out"] for c in range(NCORES)], axis=0)
    return out.astype(np.float32)


# revision 12
# speedup vs baseline: 1.5595x; 1.0633x over previous
"""CNN+GRU kernel for Trainium2, 8-core SPMD, data-parallel over batch.

Model (per reference):
  onehot(x) -> Conv1d(V=512,H=512,k=3,pad=1) -> ReLU -> GRU(H=512) -> last
  hidden -> Linear(H,C=20).   x: (B=128, L=1024) int64.

Fused single-pass design (per core, batch shard of 16):
  The sequence is processed in 32 chunks of 32 timesteps.  For chunk c the
  GRU scan (serial, latency-bound) runs while the conv+gi precompute
  ("phase A") for chunk c+1 is instruction-interleaved into the same
  engines' idle slots; gi never leaves SBUF (double-buffered per parity).
  This removes the 200MB gi DRAM roundtrip of the two-pass version and
  keeps the PE array busy enough to hold its HAM clock at 2.4 GHz.

  Scan step (all gates bf16 matmuls, h kept in bf16 only):
    - one identity matmul preloads all 8 r/z gi slices into PSUM (start)
    - one K=4 selector matmul preloads b_hh[n] into the n-gate PSUM
    - 48 whh matmul pairs accumulate W_hh @ h (r chunks, n chunks, z chunks)
    - ACT: sigmoid(r) [psum], sigmoid(z) [psum], tanh(w)
    - DVE: v=pn*r, w=v+gi_n, omz=1-z, zh=z*h, m=nt*omz, h'=m+zh (bf16 out)
  Phase A per chunk: one-hot on GPSIMD, 48 conv + 48 gi matmuls on PE
  (3-4 per scan step), ReLU on ACT, psum->SBUF gi copies (+bias) on GPSIMD.
"""

import numpy as np
import ml_dtypes
from contextlib import ExitStack

import concourse.bass as bass
import concourse.mybir as mybir
import concourse.tile as tile
from concourse import bacc
from concourse.bass import ds
from concourse.bass_utils import run_bass_kernel_spmd

F32 = mybir.dt.float32
BF16 = mybir.dt.bfloat16

B, L, V, H, C = 128, 1024, 512, 512, 20
NCORES = 8
BS = B // NCORES          # 16 batch rows per core
LBLK = 32                 # timesteps per chunk
NCHUNK = L // LBLK        # 32 chunks
W_COLS = LBLK * BS + 2 * BS   # 544: 512 positions + conv halo

Relu = mybir.ActivationFunctionType.Relu
Identity = mybir.ActivationFunctionType.Identity
Sigmoid = mybir.ActivationFunctionType.Sigmoid
Tanh = mybir.ActivationFunctionType.Tanh
EQ = mybir.AluOpType.is_equal
ADD = mybir.AluOpType.add
MULT = mybir.AluOpType.mult


def build():
    nc = bacc.Bacc(
        "TRN2", target_bir_lowering=False, debug=False, num_devices=NCORES
    )

    def din(name, shape, dt=F32):
        return nc.dram_tensor(name, list(shape), dt, kind="ExternalInput").ap()

    xpad_d = din("xpad", [(L + 2) * BS])            # l-major, sentinel rows
    wt_d = din("wt", [128, 12, 512], BF16)          # conv taps (p,[k,vc],h)
    wih_d = din("wih", [128, 4, 3 * H], BF16)       # (p, hc, g)
    whh_d = din("whh", [128, 4, 3 * H], BF16)       # (p, hc, g)
    gib_d = din("gib", [128, 12])                   # b_ih (+b_hh for r,z)
    bhnT_d = din("bhnT", [4, 128], BF16)            # b_hh n-part, [hc, p]
    sel_d = din("sel", [4, 64], BF16)               # selector for bhn bcast
    ident_d = din("ident", [128, 128], BF16)        # identity for gi preload
    convb_d = din("convb", [128, 4])
    clsw_d = din("clsw", [128, 4, C], BF16)
    clsb_d = din("clsb", [BS, C])
    iota_d = din("iota", [128, 4])
    out_d = nc.dram_tensor("out", [BS, C], F32, kind="ExternalOutput").ap()

    ET = mybir.EngineType
    hint = (ET.PE, ET.DVE, ET.Activation, ET.SP, ET.Pool)

    with tile.TileContext(nc) as tc, ExitStack() as ctx:
        singles = ctx.enter_context(tc.tile_pool(name="singles", bufs=1))

        def load_const(ap_d, name):
            t = singles.tile(list(ap_d.shape), ap_d.dtype, tag=name)
            nc.sync.dma_start(t, ap_d)
            return t

        wt_sb = load_const(wt_d, "wt")
        wih_sb = load_const(wih_d, "wih")
        whh_sb = load_const(whh_d, "whh")
        gib_sb = load_const(gib_d, "gib")
        bhnT_sb = load_const(bhnT_d, "bhnT")
        sel_sb = load_const(sel_d, "sel")
        ident_sb = load_const(ident_d, "ident")
        convb_sb = load_const(convb_d, "convb")
        clsw_sb = load_const(clsw_d, "clsw")
        clsb_sb = load_const(clsb_d, "clsb")
        iota_sb = load_const(iota_d, "iota")

        # ---- persistent state / double-buffered tiles (parity-indexed) ----
        state = ctx.enter_context(tc.tile_pool(name="state", bufs=1))
        # h in bf16 only, double-buffered per step parity
        hbf = [state.tile([128, 4, BS], BF16, name=f"hbf{i}") for i in range(2)]
        # gi buffers per chunk parity: r/z part bf16 (matmul rhs), n part f32
        girz = [state.tile([128, 8, 512], BF16, name=f"girz{i}") for i in range(2)]
        gin = [state.tile([128, 4, 512], F32, name=f"gin{i}") for i in range(2)]
        # phase-A transients, per chunk parity
        xb = [state.tile([128, W_COLS], F32, name=f"xb{i}") for i in range(2)]
        oh = [[state.tile([128, W_COLS], BF16, name=f"oh{i}_{vc}") for vc in range(4)]
              for i in range(2)]
        yt = [[state.tile([128, 512], BF16, name=f"yt{i}_{m}") for m in range(4)]
              for i in range(2)]
        # scan elementwise scratch
        rs = state.tile([128, 4, BS], F32)
        zs = state.tile([128, 4, BS], F32)
        omz = state.tile([128, 4, BS], F32)
        zh = state.tile([128, 4, BS], F32)
        vv = state.tile([128, 4, BS], F32)
        ww = state.tile([128, 4, BS], F32)
        nt = state.tile([128, 4, BS], F32)

        # PSUM pools
        psScan = ctx.enter_context(tc.tile_pool(name="psScan", bufs=2, space="PSUM"))
        psConv = ctx.enter_context(tc.tile_pool(name="psConv", bufs=1, space="PSUM"))
        psGi = ctx.enter_context(tc.tile_pool(name="psGi", bufs=1, space="PSUM"))

        nc.vector.memset(hbf[0], 0.0)

        # ---------------- phase A op-closure generator ----------------
        # Yields ("pe"|"other", closure).  "pe" closures emit tensor-engine
        # matmuls and are placed at the BOTTOM of a scan step (after the
        # step's scan matmuls, filling the PE-idle tail); "other" closures
        # emit ACT/GPSIMD/DMA ops and go at the TOP of a step so they never
        # sit in front of the step's critical ACT work in queue order.
        # FIFO order is preserved so producers always precede consumers.
        def phase_a_ops(p, xoff):
            def dma_x():
                nc.sync.dma_start(
                    xb[p], xpad_d[ds(xoff, W_COLS)].partition_broadcast(128)
                )
            yield "other", dma_x
            for vc in range(4):
                def onehot(vc=vc):
                    nc.vector.tensor_scalar(
                        oh[p][vc], xb[p], iota_sb[:, vc : vc + 1], None, EQ
                    )
                yield "other", onehot
            # conv: 4 output chunks x 12 accum matmuls, 3 per closure
            conv_ps = [None] * 4
            for m in range(4):
                for part in range(4):
                    def conv_mm(m=m, part=part):
                        if part == 0:
                            conv_ps[m] = psConv.tile([128, 512], F32, name="cps", tag="cps")
                        for i in range(3):
                            j = part * 3 + i
                            k, vc = j // 4, j % 4
                            nc.tensor.matmul(
                                conv_ps[m],
                                wt_sb[:, k * 4 + vc, m * 128 : (m + 1) * 128],
                                oh[p][vc][:, k * BS : k * BS + 512],
                                start=(j == 0),
                                stop=(j == 11),
                            )
                    yield "pe", conv_mm
                def relu(m=m):
                    nc.scalar.activation(
                        yt[p][m], conv_ps[m], Relu, bias=convb_sb[:, m : m + 1]
                    )
                yield "other", relu
            # gi: 12 gate chunks x 4 accum matmuls + gpsimd copy w/ bias
            gi_ps = [None] * 12
            for g in range(12):
                def gi_mm(g=g):
                    gi_ps[g] = psGi.tile([128, 512], F32, name="gps", tag="gps")
                    for hc in range(4):
                        nc.tensor.matmul(
                            gi_ps[g],
                            wih_sb[:, hc, g * 128 : (g + 1) * 128],
                            yt[p][hc],
                            start=(hc == 0),
                            stop=(hc == 3),
                        )
                yield "pe", gi_mm
                def gi_copy(g=g):
                    if g < 8:
                        dst = girz[p][:, g]
                    else:
                        dst = gin[p][:, g - 8]
                    nc.vector.tensor_scalar(
                        dst, gi_ps[g], gib_sb[:, g : g + 1], None, ADD
                    )
                yield "other", gi_copy

        class PADispatch:
            """Strict-FIFO dispatcher: per scan step, emit leading 'other'
            closures at the step top (max 2) and then, after the scan
            matmuls, leading 'pe' closures (max 1)."""
            def __init__(self, gen):
                self.pend = list(gen) if gen is not None else []
                self.i = 0

            def top(self):
                n = 0
                while self.i < len(self.pend) and n < 2 \
                        and self.pend[self.i][0] == "other":
                    self.pend[self.i][1]()
                    self.i += 1
                    n += 1

            def bottom(self):
                if self.i < len(self.pend) and self.pend[self.i][0] == "pe":
                    self.pend[self.i][1]()
                    self.i += 1

            def drain(self):
                while self.i < len(self.pend):
                    self.pend[self.i][1]()
                    self.i += 1

        # ---------------- scan step ----------------
        def emit_preload(p_gi, s):
            """Allocate psum tiles for step s and emit its gi/bhn preload
            matmuls (no h dependency, so they run during the previous
            step's elementwise tail)."""
            pr = psScan.tile([128, 4, BS], F32, name="pr", tag="pr")
            pz = psScan.tile([128, 4, BS], F32, name="pz", tag="pz")
            pn = psScan.tile([128, 4, BS], F32, name="pn", tag="pn")
            nc.tensor.matmul(
                pr, ident_sb, girz[p_gi][:, 0:4, s * BS : (s + 1) * BS],
                start=True, stop=False,
            )
            nc.tensor.matmul(
                pz, ident_sb, girz[p_gi][:, 4:8, s * BS : (s + 1) * BS],
                start=True, stop=False,
            )
            nc.tensor.matmul(pn, bhnT_sb, sel_sb, start=True, stop=False)
            return pr, pz, pn

        def scan_step(p_gi, s, hb_in, hb_out, pa, cur, nxt_preload):
            pa.top()
            pr, pz, pn = cur
            # whh accumulation: r chunks, z chunks, n chunks
            for g in range(12):
                dst = pr[:, g] if g < 4 else (pz[:, g - 4] if g < 8 else pn[:, g - 8])
                for hc in range(4):
                    nc.tensor.matmul(
                        dst,
                        whh_sb[:, hc, g * 128 : (g + 1) * 128],
                        hb_in[:, hc],
                        start=False,
                        stop=(hc == 3),
                    )
            # next step's preloads go right behind this step's matmuls
            nxt = nxt_preload() if nxt_preload is not None else None
            # interleave phase-A PE work for the next chunk into this step
            pa.bottom()
            # elementwise chain
            nc.scalar.activation(rs, pr, Sigmoid)
            nc.scalar.activation(omz, pz, Sigmoid, scale=-1.0)   # 1-z
            nc.scalar.activation(zs, pz, Sigmoid)
            nc.vector.tensor_mul(vv, pn, rs)
            nc.vector.tensor_add(ww, vv, gin[p_gi][:, :, s * BS : (s + 1) * BS])
            nc.scalar.activation(nt, ww, Tanh)
            nc.vector.tensor_mul(zh, zs, hb_in)
            nc.vector.tensor_mul(vv, nt, omz)          # reuse vv as m
            nc.vector.tensor_add(hb_out, vv, zh)       # bf16 h'
            return nxt

        def scan_chunk(c_par, pa, cur, nxt_chunk_preload=None):
            """cur = preloaded psum tiles for step 0 (from emit_preload).
            Returns the preload tiles for the next chunk's step 0 if
            nxt_chunk_preload was given."""
            for s in range(LBLK):
                if s < LBLK - 1:
                    nxt_fn = lambda s=s: emit_preload(c_par, s + 1)
                else:
                    nxt_fn = nxt_chunk_preload
                cur = scan_step(c_par, s, hbf[s % 2], hbf[1 - s % 2], pa,
                                cur, nxt_fn)
            return cur

        # ---------------- prologue: phase A for chunk 0 ----------------
        PADispatch(phase_a_ops(0, 0)).drain()

        # ---------------- main loop ----------------
        # chunk pair (2t, 2t+1); phase A for chunks 2t+1, 2t+2.
        # trips cover scan chunks 0..29, phase A 1..30;
        # tail: scan 30 + phase A 31, then scan 31.
        with tc.For_i(0, 30, 2, hint_engines=hint) as t2:
            paA = PADispatch(phase_a_ops(1, (t2 + 1) * (LBLK * BS)))
            cur = emit_preload(0, 0)
            cur = scan_chunk(0, paA, cur, lambda: emit_preload(1, 0))
            paA.drain()
            paB = PADispatch(phase_a_ops(0, (t2 + 2) * (LBLK * BS)))
            scan_chunk(1, paB, cur)
            paB.drain()

        # tail: scan chunk 30 (parity 0), phase A chunk 31 -> buf 1
        paT = PADispatch(phase_a_ops(1, 31 * (LBLK * BS)))
        cur = emit_preload(0, 0)
        cur = scan_chunk(0, paT, cur, lambda: emit_preload(1, 0))
        paT.drain()
        # scan chunk 31 (parity 1), no phase A
        scan_chunk(1, PADispatch(None), cur)

        # ---------------- classifier ----------------
        pc = psGi.tile([BS, C], F32, name="pc", tag="gps",
                       padded_shape=[BS, 512])
        for hc in range(4):
            nc.tensor.matmul(
                pc, hbf[0][:, hc], clsw_sb[:, hc],
                start=(hc == 0), stop=(hc == 3),
            )
        outs = singles.tile([BS, C], F32)
        nc.vector.tensor_add(outs, pc, clsb_sb)
        nc.sync.dma_start(out_d, outs)

    nc.compile()
    return nc


def host_prep(x, conv_w, conv_b, w_ih, w_hh, b_ih, b_hh, cls_w, cls_b):
    x = np.asarray(x)
    conv_w = np.asarray(conv_w, np.float32)
    conv_b = np.asarray(conv_b, np.float32)
    w_ih = np.asarray(w_ih, np.float32)
    w_hh = np.asarray(w_hh, np.float32)
    b_ih = np.asarray(b_ih, np.float32)
    b_hh = np.asarray(b_hh, np.float32)
    cls_w = np.asarray(cls_w, np.float32)
    cls_b = np.asarray(cls_b, np.float32)
    bf16 = ml_dtypes.bfloat16

    # conv taps: wt[p, k*4+vc, h] = conv_w[h, vc*128+p, k]
    Wv = conv_w.transpose(1, 0, 2)                    # (V, H, 3)
    wt = np.ascontiguousarray(
        Wv.reshape(4, 128, H, 3).transpose(1, 3, 0, 2).reshape(128, 12, H)
    ).astype(bf16)
    wih = np.ascontiguousarray(
        w_ih.T.reshape(4, 128, 3 * H).transpose(1, 0, 2)
    ).astype(bf16)
    whh = np.ascontiguousarray(
        w_hh.T.reshape(4, 128, 3 * H).transpose(1, 0, 2)
    ).astype(bf16)
    bb = b_ih.copy()
    bb[: 2 * H] += b_hh[: 2 * H]
    gib = np.ascontiguousarray(bb.reshape(12, 128).T)
    bhnT = np.ascontiguousarray(b_hh[2 * H :].reshape(4, 128)).astype(bf16)
    sel = np.zeros((4, 4, BS), np.float32)
    for k in range(4):
        sel[k, k, :] = 1.0
    sel = sel.reshape(4, 64).astype(bf16)
    ident = np.eye(128, dtype=np.float32).astype(bf16)
    convb = np.ascontiguousarray(conv_b.reshape(4, 128).T)
    clsw = np.ascontiguousarray(
        cls_w.T.reshape(4, 128, C).transpose(1, 0, 2)
    ).astype(bf16)
    clsb = np.tile(cls_b[None, :], (BS, 1)).astype(np.float32)
    iota = np.ascontiguousarray(np.arange(V, dtype=np.float32).reshape(4, 128).T)

    shared = {
        "wt": wt, "wih": wih, "whh": whh, "gib": gib, "bhnT": bhnT,
        "sel": sel, "ident": ident, "convb": convb, "clsw": clsw,
        "clsb": clsb, "iota": iota,
    }
    in_maps = []
    for c in range(NCORES):
        xc = x[c * BS : (c + 1) * BS, :].astype(np.float32).T  # (L, BS)
        xpad = np.full((L + 2, BS), float(V), np.float32)
        xpad[1:-1] = xc
        in_maps.append({**shared, "xpad": np.ascontiguousarray(xpad.ravel())})
    return in_maps


_BUILT = {}


def _get_nc():
    if "nc" not in _BUILT:
        _BUILT["nc"] = build()
    return _BUILT["nc"]


def kernel(x, conv_w, conv_b, w_ih, w_hh, b_ih, b_hh, cls_w, cls_b):
    nc = _get_nc()
    in_maps = host_prep(
        x, conv_w, conv_b, w_ih, w_hh, b_ih, b_hh, cls_w, cls_b
    )
    res = run_bass_kernel_spmd(nc, in_maps, core_ids=list(range(NCORES)))
    out = np.concatenate([res.results[c]["out"] for c in range(NCORES)], axis=0)
    return out.astype(np.float32)


# revision 13
# speedup vs baseline: 1.5728x; 1.0085x over previous
"""CNN+GRU kernel for Trainium2, 8-core SPMD, data-parallel over batch.

Model (per reference):
  onehot(x) -> Conv1d(V=512,H=512,k=3,pad=1) -> ReLU -> GRU(H=512) -> last
  hidden -> Linear(H,C=20).   x: (B=128, L=1024) int64.

Fused single-pass design (per core, batch shard of 16):
  The sequence is processed in 32 chunks of 32 timesteps.  For chunk c the
  GRU scan (serial, latency-bound) runs while the conv+gi precompute
  ("phase A") for chunk c+1 is instruction-interleaved into the same
  engines' idle slots; gi never leaves SBUF (double-buffered per parity).
  This removes the 200MB gi DRAM roundtrip of the two-pass version and
  keeps the PE array busy enough to hold its HAM clock at 2.4 GHz.

  Scan step (all gates bf16 matmuls, h kept in bf16 only):
    - one identity matmul preloads all 8 r/z gi slices into PSUM (start)
    - one K=4 selector matmul preloads b_hh[n] into the n-gate PSUM
    - 48 whh matmul pairs accumulate W_hh @ h (r chunks, n chunks, z chunks)
    - ACT: sigmoid(r) [psum], sigmoid(z) [psum], tanh(w)
    - DVE: v=pn*r, w=v+gi_n, omz=1-z, zh=z*h, m=nt*omz, h'=m+zh (bf16 out)
  Phase A per chunk: one-hot on GPSIMD, 48 conv + 48 gi matmuls on PE
  (3-4 per scan step), ReLU on ACT, psum->SBUF gi copies (+bias) on GPSIMD.
"""

import numpy as np
import ml_dtypes
from contextlib import ExitStack

import concourse.bass as bass
import concourse.mybir as mybir
import concourse.tile as tile
from concourse import bacc
from concourse.bass import ds
from concourse.bass_utils import run_bass_kernel_spmd

F32 = mybir.dt.float32
BF16 = mybir.dt.bfloat16

B, L, V, H, C = 128, 1024, 512, 512, 20
NCORES = 8
BS = B // NCORES          # 16 batch rows per core
LBLK = 32                 # timesteps per chunk
NCHUNK = L // LBLK        # 32 chunks
W_COLS = LBLK * BS + 2 * BS   # 544: 512 positions + conv halo

Relu = mybir.ActivationFunctionType.Relu
Identity = mybir.ActivationFunctionType.Identity
Sigmoid = mybir.ActivationFunctionType.Sigmoid
Tanh = mybir.ActivationFunctionType.Tanh
EQ = mybir.AluOpType.is_equal
ADD = mybir.AluOpType.add
MULT = mybir.AluOpType.mult


def build():
    nc = bacc.Bacc(
        "TRN2", target_bir_lowering=False, debug=False, num_devices=NCORES
    )

    def din(name, shape, dt=F32):
        return nc.dram_tensor(name, list(shape), dt, kind="ExternalInput").ap()

    xpad_d = din("xpad", [(L + 2) * BS])            # l-major, sentinel rows
    wt_d = din("wt", [128, 12, 512], BF16)          # conv taps (p,[k,vc],h)
    wih_d = din("wih", [128, 4, 3 * H], BF16)       # (p, hc, g)
    whh_d = din("whh", [128, 4, 3 * H], BF16)       # (p, hc, g)
    gib_d = din("gib", [128, 12])                   # b_ih (+b_hh for r,z)
    bhnT_d = din("bhnT", [4, 128], BF16)            # b_hh n-part, [hc, p]
    sel_d = din("sel", [4, 64], BF16)               # selector for bhn bcast
    ident_d = din("ident", [128, 128], BF16)        # identity for gi preload
    convb_d = din("convb", [128, 4])
    clsw_d = din("clsw", [128, 4, C], BF16)
    clsb_d = din("clsb", [BS, C])
    iota_d = din("iota", [128, 4])
    out_d = nc.dram_tensor("out", [BS, C], F32, kind="ExternalOutput").ap()

    ET = mybir.EngineType
    hint = (ET.PE, ET.DVE, ET.Activation, ET.SP, ET.Pool)

    with tile.TileContext(nc) as tc, ExitStack() as ctx:
        singles = ctx.enter_context(tc.tile_pool(name="singles", bufs=1))

        def load_const(ap_d, name):
            t = singles.tile(list(ap_d.shape), ap_d.dtype, tag=name)
            nc.sync.dma_start(t, ap_d)
            return t

        wt_sb = load_const(wt_d, "wt")
        wih_sb = load_const(wih_d, "wih")
        whh_sb = load_const(whh_d, "whh")
        gib_sb = load_const(gib_d, "gib")
        bhnT_sb = load_const(bhnT_d, "bhnT")
        sel_sb = load_const(sel_d, "sel")
        ident_sb = load_const(ident_d, "ident")
        convb_sb = load_const(convb_d, "convb")
        clsw_sb = load_const(clsw_d, "clsw")
        clsb_sb = load_const(clsb_d, "clsb")
        iota_sb = load_const(iota_d, "iota")

        # ---- persistent state / double-buffered tiles (parity-indexed) ----
        state = ctx.enter_context(tc.tile_pool(name="state", bufs=1))
        # h in bf16 only, double-buffered per step parity
        hbf = [state.tile([128, 4, BS], BF16, name=f"hbf{i}") for i in range(2)]
        # gi buffers per chunk parity: r/z part bf16 (matmul rhs), n part f32
        girz = [state.tile([128, 8, 512], BF16, name=f"girz{i}") for i in range(2)]
        gin = [state.tile([128, 4, 512], F32, name=f"gin{i}") for i in range(2)]
        # phase-A transients, per chunk parity
        xb = [state.tile([128, W_COLS], F32, name=f"xb{i}") for i in range(2)]
        oh = [[state.tile([128, W_COLS], BF16, name=f"oh{i}_{vc}") for vc in range(4)]
              for i in range(2)]
        yt = [[state.tile([128, 512], BF16, name=f"yt{i}_{m}") for m in range(4)]
              for i in range(2)]
        # scan elementwise scratch
        rs = state.tile([128, 4, BS], F32)
        zs = state.tile([128, 4, BS], F32)
        omz = state.tile([128, 4, BS], F32)
        zh = state.tile([128, 4, BS], F32)
        vv = state.tile([128, 4, BS], F32)
        ww = state.tile([128, 4, BS], F32)
        nt = state.tile([128, 4, BS], F32)

        # PSUM pools
        psScan = ctx.enter_context(tc.tile_pool(name="psScan", bufs=2, space="PSUM"))
        psConv = ctx.enter_context(tc.tile_pool(name="psConv", bufs=1, space="PSUM"))
        psGi = ctx.enter_context(tc.tile_pool(name="psGi", bufs=1, space="PSUM"))

        nc.vector.memset(hbf[0], 0.0)

        # ---------------- phase A op-closure generator ----------------
        # Yields ("pe"|"other", closure).  "pe" closures emit tensor-engine
        # matmuls and are placed at the BOTTOM of a scan step (after the
        # step's scan matmuls, filling the PE-idle tail); "other" closures
        # emit ACT/GPSIMD/DMA ops and go at the TOP of a step so they never
        # sit in front of the step's critical ACT work in queue order.
        # FIFO order is preserved so producers always precede consumers.
        def phase_a_ops(p, xoff):
            def dma_x():
                nc.sync.dma_start(
                    xb[p], xpad_d[ds(xoff, W_COLS)].partition_broadcast(128)
                )
            yield "other", dma_x
            for vc in range(4):
                def onehot(vc=vc):
                    nc.vector.tensor_scalar(
                        oh[p][vc], xb[p], iota_sb[:, vc : vc + 1], None, EQ
                    )
                yield "other", onehot
            # conv: 4 output chunks x 12 accum matmuls, 3 per closure
            conv_ps = [None] * 4
            for m in range(4):
                for part in range(6):
                    def conv_mm(m=m, part=part):
                        if part == 0:
                            conv_ps[m] = psConv.tile([128, 512], F32, name="cps", tag="cps")
                        for i in range(2):
                            j = part * 2 + i
                            k, vc = j // 4, j % 4
                            nc.tensor.matmul(
                                conv_ps[m],
                                wt_sb[:, k * 4 + vc, m * 128 : (m + 1) * 128],
                                oh[p][vc][:, k * BS : k * BS + 512],
                                start=(j == 0),
                                stop=(j == 11),
                            )
                    yield "pe", conv_mm
                def relu(m=m):
                    nc.scalar.activation(
                        yt[p][m], conv_ps[m], Relu, bias=convb_sb[:, m : m + 1]
                    )
                yield "other", relu
            # gi: 12 gate chunks x 4 accum matmuls + gpsimd copy w/ bias
            gi_ps = [None] * 12
            for g in range(12):
                for half in range(2):
                    def gi_mm(g=g, half=half):
                        if half == 0:
                            gi_ps[g] = psGi.tile([128, 512], F32, name="gps", tag="gps")
                        for hc in (2 * half, 2 * half + 1):
                            nc.tensor.matmul(
                                gi_ps[g],
                                wih_sb[:, hc, g * 128 : (g + 1) * 128],
                                yt[p][hc],
                                start=(hc == 0),
                                stop=(hc == 3),
                            )
                    yield "pe", gi_mm
                def gi_copy(g=g):
                    if g < 8:
                        dst = girz[p][:, g]
                    else:
                        dst = gin[p][:, g - 8]
                    nc.vector.tensor_scalar(
                        dst, gi_ps[g], gib_sb[:, g : g + 1], None, ADD
                    )
                yield "other", gi_copy

        class PADispatch:
            """Strict-FIFO dispatcher: per scan step, emit leading 'other'
            closures at the step top (max 2) and then, after the scan
            matmuls, leading 'pe' closures (max 1)."""
            def __init__(self, gen):
                self.pend = list(gen) if gen is not None else []
                self.i = 0

            def top(self):
                n = 0
                while self.i < len(self.pend) and n < 2 \
                        and self.pend[self.i][0] == "other":
                    self.pend[self.i][1]()
                    self.i += 1
                    n += 1

            def bottom(self):
                n = 0
                while self.i < len(self.pend) and n < 2 \
                        and self.pend[self.i][0] == "pe":
                    self.pend[self.i][1]()
                    self.i += 1
                    n += 1
                return n

            def drain(self):
                while self.i < len(self.pend):
                    self.pend[self.i][1]()
                    self.i += 1

        # ---------------- scan step ----------------
        def emit_preload(p_gi, s):
            """Allocate psum tiles for step s and emit its gi/bhn preload
            matmuls (no h dependency, so they run during the previous
            step's elementwise tail)."""
            pr = psScan.tile([128, 4, BS], F32, name="pr", tag="pr")
            pz = psScan.tile([128, 4, BS], F32, name="pz", tag="pz")
            pn = psScan.tile([128, 4, BS], F32, name="pn", tag="pn")
            nc.tensor.matmul(
                pr, ident_sb, girz[p_gi][:, 0:4, s * BS : (s + 1) * BS],
                start=True, stop=False,
            )
            nc.tensor.matmul(
                pz, ident_sb, girz[p_gi][:, 4:8, s * BS : (s + 1) * BS],
                start=True, stop=False,
            )
            nc.tensor.matmul(pn, bhnT_sb, sel_sb, start=True, stop=False)
            return pr, pz, pn

        def scan_step(p_gi, s, hb_in, hb_out, pa, cur, nxt_preload):
            pa.top()
            pr, pz, pn = cur
            # whh accumulation: r chunks, z chunks, n chunks
            for g in range(12):
                dst = pr[:, g] if g < 4 else (pz[:, g - 4] if g < 8 else pn[:, g - 8])
                for hc in range(4):
                    nc.tensor.matmul(
                        dst,
                        whh_sb[:, hc, g * 128 : (g + 1) * 128],
                        hb_in[:, hc],
                        start=False,
                        stop=(hc == 3),
                    )
            # next step's preloads go right behind this step's matmuls
            nxt = nxt_preload() if nxt_preload is not None else None
            # interleave phase-A PE work for the next chunk into this step;
            # if none available, keep the PE array hot with throwaway
            # 512-wide matmuls so HAM holds the 2.4 GHz clock
            if pa.bottom() == 0:
                dmy = psConv.tile([128, 512], F32, name="dmy", tag="cps")
                nc.tensor.matmul(dmy, ident_sb, girz[p_gi][:, 0],
                                 start=True, stop=True)
                nc.tensor.matmul(dmy, ident_sb, girz[p_gi][:, 1],
                                 start=True, stop=True)
            # elementwise chain
            nc.scalar.activation(rs, pr, Sigmoid)
            nc.scalar.activation(omz, pz, Sigmoid, scale=-1.0)   # 1-z
            nc.scalar.activation(zs, pz, Sigmoid)
            nc.vector.tensor_mul(vv, pn, rs)
            nc.vector.tensor_add(ww, vv, gin[p_gi][:, :, s * BS : (s + 1) * BS])
            nc.scalar.activation(nt, ww, Tanh)
            nc.vector.tensor_mul(zh, zs, hb_in)
            nc.vector.tensor_mul(vv, nt, omz)          # reuse vv as m
            nc.vector.tensor_add(hb_out, vv, zh)       # bf16 h'
            return nxt

        def scan_chunk(c_par, pa, cur, nxt_chunk_preload=None):
            """cur = preloaded psum tiles for step 0 (from emit_preload).
            Returns the preload tiles for the next chunk's step 0 if
            nxt_chunk_preload was given."""
            for s in range(LBLK):
                if s < LBLK - 1:
                    nxt_fn = lambda s=s: emit_preload(c_par, s + 1)
                else:
                    nxt_fn = nxt_chunk_preload
                cur = scan_step(c_par, s, hbf[s % 2], hbf[1 - s % 2], pa,
                                cur, nxt_fn)
            return cur

        # ---------------- prologue: phase A for chunk 0 ----------------
        PADispatch(phase_a_ops(0, 0)).drain()

        # ---------------- main loop ----------------
        # chunk pair (2t, 2t+1); phase A for chunks 2t+1, 2t+2.
        # trips cover scan chunks 0..29, phase A 1..30;
        # tail: scan 30 + phase A 31, then scan 31.
        with tc.For_i(0, 30, 2, hint_engines=hint) as t2:
            paA = PADispatch(phase_a_ops(1, (t2 + 1) * (LBLK * BS)))
            cur = emit_preload(0, 0)
            cur = scan_chunk(0, paA, cur, lambda: emit_preload(1, 0))
            paA.drain()
            paB = PADispatch(phase_a_ops(0, (t2 + 2) * (LBLK * BS)))
            scan_chunk(1, paB, cur)
            paB.drain()

        # tail: scan chunk 30 (parity 0), phase A chunk 31 -> buf 1
        paT = PADispatch(phase_a_ops(1, 31 * (LBLK * BS)))
        cur = emit_preload(0, 0)
        cur = scan_chunk(0, paT, cur, lambda: emit_preload(1, 0))
        paT.drain()
        # scan chunk 31 (parity 1), no phase A
        scan_chunk(1, PADispatch(None), cur)

        # ---------------- classifier ----------------
        pc = psGi.tile([BS, C], F32, name="pc", tag="gps",
                       padded_shape=[BS, 512])
        for hc in range(4):
            nc.tensor.matmul(
                pc, hbf[0][:, hc], clsw_sb[:, hc],
                start=(hc == 0), stop=(hc == 3),
            )
        outs = singles.tile([BS, C], F32)
        nc.vector.tensor_add(outs, pc, clsb_sb)
        nc.sync.dma_start(out_d, outs)

    nc.compile()
    return nc


def host_prep(x, conv_w, conv_b, w_ih, w_hh, b_ih, b_hh, cls_w, cls_b):
    x = np.asarray(x)
    conv_w = np.asarray(conv_w, np.float32)
    conv_b = np.asarray(conv_b, np.float32)
    w_ih = np.asarray(w_ih, np.float32)
    w_hh = np.asarray(w_hh, np.float32)
    b_ih = np.asarray(b_ih, np.float32)
    b_hh = np.asarray(b_hh, np.float32)
    cls_w = np.asarray(cls_w, np.float32)
    cls_b = np.asarray(cls_b, np.float32)
    bf16 = ml_dtypes.bfloat16

    # conv taps: wt[p, k*4+vc, h] = conv_w[h, vc*128+p, k]
    Wv = conv_w.transpose(1, 0, 2)                    # (V, H, 3)
    wt = np.ascontiguousarray(
        Wv.reshape(4, 128, H, 3).transpose(1, 3, 0, 2).reshape(128, 12, H)
    ).astype(bf16)
    wih = np.ascontiguousarray(
        w_ih.T.reshape(4, 128, 3 * H).transpose(1, 0, 2)
    ).astype(bf16)
    whh = np.ascontiguousarray(
        w_hh.T.reshape(4, 128, 3 * H).transpose(1, 0, 2)
    ).astype(bf16)
    bb = b_ih.copy()
    bb[: 2 * H] += b_hh[: 2 * H]
    gib = np.ascontiguousarray(bb.reshape(12, 128).T)
    bhnT = np.ascontiguousarray(b_hh[2 * H :].reshape(4, 128)).astype(bf16)
    sel = np.zeros((4, 4, BS), np.float32)
    for k in range(4):
        sel[k, k, :] = 1.0
    sel = sel.reshape(4, 64).astype(bf16)
    ident = np.eye(128, dtype=np.float32).astype(bf16)
    convb = np.ascontiguousarray(conv_b.reshape(4, 128).T)
    clsw = np.ascontiguousarray(
        cls_w.T.reshape(4, 128, C).transpose(1, 0, 2)
    ).astype(bf16)
    clsb = np.tile(cls_b[None, :], (BS, 1)).astype(np.float32)
    iota = np.ascontiguousarray(np.arange(V, dtype=np.float32).reshape(4, 128).T)

    shared = {
        "wt": wt, "wih": wih, "whh": whh, "gib": gib, "bhnT": bhnT,
        "sel": sel, "ident": ident, "convb": convb, "clsw": clsw,
        "clsb": clsb, "iota": iota,
    }
    in_maps = []
    for c in range(NCORES):
        xc = x[c * BS : (c + 1) * BS, :].astype(np.float32).T  # (L, BS)
        xpad = np.full((L + 2, BS), float(V), np.float32)
        xpad[1:-1] = xc
        in_maps.append({**shared, "xpad": np.ascontiguousarray(xpad.ravel())})
    return in_maps


_BUILT = {}


def _get_nc():
    if "nc" not in _BUILT:
        _BUILT["nc"] = build()
    return _BUILT["nc"]


def kernel(x, conv_w, conv_b, w_ih, w_hh, b_ih, b_hh, cls_w, cls_b):
    nc = _get_nc()
    in_maps = host_prep(
        x, conv_w, conv_b, w_ih, w_hh, b_ih, b_hh, cls_w, cls_b
    )
    res = run_bass_kernel_spmd(nc, in_maps, core_ids=list(range(NCORES)))
    out = np.concatenate([res.results[c]["out"] for c in range(NCORES)], axis=0)
    return out.astype(np.float32)
